# revision 12
# baseline (speedup 1.0000x reference)
"""EnergyStatistics segment-reduce kernel for 8x TRN2 NeuronCores.

Strategy: batch-shard the 32768 rows across 8 cores (4096 rows each, all 32
configs per core). Per-cluster sums/counts are computed with one-hot matmuls
on the tensor engine, AllReduce'd across cores, then a second pass computes
per-sample distances to assigned centroids fully on-device:

  pass A : St[d, (c,k)] = sum_i f[i,d] * oh_c[i,k]     (PE, fp16 streams)
           counts[(c,k)] = sum_i oh_c[i,k]
  AR1    : AllReduce [129, 3200] partials
  mid    : Ct = St/max(counts,1);  cn2 = ||Ct||^2 per column
  pass B : G'[i,(c,k)] = f_i . c_k - cn2_k/2            (PE)
           DST = sqrt(-2*G' + |f_i|^2)  = ||f_i - c_k|| (ACT, all pairs)
           per_sum[(c,k)] = sum_i oh * DST              (DVE mask + PE colsum)
  AR2    : AllReduce [1, 3200]
  final  : entropy/h_a/h_r/delta per config + eval-mode normalization,
           computed redundantly on each core; each core writes its own
           [4096, 32, 4] slice of the output.
"""

import numpy as np
from contextlib import ExitStack

import concourse.bass as bass
import concourse.bacc as bacc
import concourse.tile as tile
import concourse.mybir as mybir
from concourse.bass_utils import run_bass_kernel_spmd

F32 = mybir.dt.float32
F16 = mybir.dt.float16
I32 = mybir.dt.int32
I16 = mybir.dt.int16
ALU = mybir.AluOpType
ACTF = mybir.ActivationFunctionType

B, D, NC, K = 32768, 128, 32, 100
KC = NC * K            # 3200
NCG = 16               # configs per group (psum capacity)
KCG = NCG * K          # 1600
NG = NC // NCG         # 2
BIG = 1e30
P = 128


def _chunks(total, width=512):
    o = 0
    while o < total:
        w = min(width, total - o)
        yield o, w
        o += w


def _emit(tc, ctx, n_cores, BL, q_eps=0.0):
    nc = tc.nc
    T = BL // P

    feat_d = nc.dram_tensor("features", [BL, D], F32, kind="ExternalInput")
    assign_d = nc.dram_tensor("assign", [BL, NC], I32, kind="ExternalInput")
    rm_d = nc.dram_tensor("rmean", [NC, 4], F32, kind="ExternalInput")
    rv_d = nc.dram_tensor("rvar", [NC, 4], F32, kind="ExternalInput")
    out_d = nc.dram_tensor("out", [BL, NC * 4], F32, kind="ExternalOutput")

    const = ctx.enter_context(tc.tile_pool(name="const", bufs=1))
    big = ctx.enter_context(tc.tile_pool(name="big", bufs=1))
    rows = ctx.enter_context(tc.tile_pool(name="rows", bufs=1))
    rows2 = ctx.enter_context(tc.tile_pool(name="rows2", bufs=2))
    ohp = ctx.enter_context(tc.tile_pool(name="ohp", bufs=3))
    scr = ctx.enter_context(tc.tile_pool(name="scr", bufs=2))
    fin = ctx.enter_context(tc.tile_pool(name="fin", bufs=1))
    dram = ctx.enter_context(tc.tile_pool(name="dramp", bufs=1, space="DRAM"))

    # ---- constants -------------------------------------------------------
    iota_i = const.tile([P, K], I16)
    nc.gpsimd.iota(iota_i[:], [[1, K]], channel_multiplier=0)
    ik16 = const.tile([P, K], F16)
    nc.vector.tensor_copy(ik16[:], iota_i[:])

    irow_i = const.tile([P, P], I16)
    nc.gpsimd.iota(irow_i[:], [[1, P]], channel_multiplier=0)
    irow16 = const.tile([P, P], F16)
    nc.vector.tensor_copy(irow16[:], irow_i[:])
    icol_i = const.tile([P, 1], I16)
    nc.gpsimd.iota(icol_i[:], [[0, 1]], channel_multiplier=1)
    icol_f = const.tile([P, 1], F32)
    nc.vector.tensor_copy(icol_f[:], icol_i[:])
    ident16 = const.tile([P, P], F16)
    nc.vector.tensor_scalar(
        out=ident16[:], in0=irow16[:], scalar1=icol_f[:, 0:1], scalar2=None,
        op0=ALU.is_equal)
    ident32 = const.tile([P, P], F32)
    nc.vector.tensor_copy(ident32[:], ident16[:])

    ones_col16 = const.tile([P, 1], F16)
    nc.vector.memset(ones_col16[:], 1.0)
    ones_row16 = const.tile([1, P], F16)
    nc.vector.memset(ones_row16[:], 1.0)
    ones_row32 = const.tile([1, P], F32)
    nc.vector.memset(ones_row32[:], 1.0)

    # tri16[k, k'] = 1 if k < k' < K else 0   (shape [P, P], rows>=K unused)
    tri16 = const.tile([P, P], F16)
    t_gt = const.tile([P, P], F16)
    nc.vector.tensor_scalar(
        out=t_gt[:], in0=irow16[:], scalar1=icol_f[:, 0:1], scalar2=None,
        op0=ALU.is_gt)
    t_lt = const.tile([P, P], F16)
    nc.vector.tensor_scalar(
        out=t_lt[:], in0=irow16[:], scalar1=float(K), scalar2=None,
        op0=ALU.is_lt)
    nc.vector.tensor_tensor(out=tri16[:], in0=t_gt[:], in1=t_lt[:], op=ALU.mult)

    # ---- load inputs (tile-wise staging keeps SBUF small) ----------------
    f16t = big.tile([P, T * D], F16)
    aft = big.tile([P, T * NC], F32)
    fnorm = big.tile([P, T], F32)
    fview = feat_d.ap().rearrange("(n p) d -> p n d", p=P)
    aview = assign_d.ap().rearrange("(n p) c -> p n c", p=P)
    for n in range(T):
        fl = scr.tile([P, D], F32, tag="fload")
        nc.sync.dma_start(out=fl[:], in_=fview[:, n])
        nc.vector.tensor_copy(f16t[:, n * D:(n + 1) * D], fl[:])
        sq = scr.tile([P, D], F16, tag="sqscr")
        nc.scalar.activation(out=sq[:], in_=fl[:], func=ACTF.Square,
                             accum_out=fnorm[:, n:n + 1])
        al = scr.tile([P, NC], I32, tag="aload")
        nc.sync.dma_start(out=al[:], in_=aview[:, n])
        nc.vector.tensor_copy(aft[:, n * NC:(n + 1) * NC], al[:])
    if q_eps:
        nc.vector.tensor_scalar(out=fnorm[:], in0=fnorm[:], scalar1=q_eps,
                                scalar2=None, op0=ALU.add)

    # f transposed (d on partitions), via PE transpose
    fT16 = big.tile([P, T * D], F16)
    with tc.tile_pool(name="psT", bufs=2, space="PSUM") as psT:
        for n in range(T):
            pst = psT.tile([P, D], F16, tag="pst")
            nc.tensor.transpose(pst[:], f16t[:, n * D:(n + 1) * D], ident16[:])
            nc.scalar.copy(fT16[:, n * D:(n + 1) * D], pst[:])

    def gen_oh(n, g):
        oh = ohp.tile([P, KCG], F16, tag="oh")
        for j in range(NCG):
            c = g * NCG + j
            nc.vector.tensor_scalar(
                out=oh[:, j * K:(j + 1) * K], in0=ik16[:],
                scalar1=aft[:, n * NC + c:n * NC + c + 1], scalar2=None,
                op0=ALU.is_equal)
        return oh

    # ---- pass A: segment sums + counts ----------------------------------
    st32 = big.tile([P, KC], F32)
    counts = rows.tile([1, KC], F32)
    with tc.tile_pool(name="psA", bufs=1, space="PSUM") as psA:
        for g in range(NG):
            St = psA.tile([P, KCG], F32, tag="st")
            Cnt = psA.tile([1, KCG], F32, tag="cnt")
            for n in range(T):
                oh = gen_oh(n, g)
                fst = f16t[:, n * D:(n + 1) * D]
                for o, w in _chunks(KCG):
                    nc.tensor.matmul(St[:, o:o + w], fst, oh[:, o:o + w],
                                     start=(n == 0), stop=(n == T - 1))
                    nc.tensor.matmul(Cnt[:, o:o + w], ones_col16[:],
                                     oh[:, o:o + w],
                                     start=(n == 0), stop=(n == T - 1))
            gs = slice(g * KCG, (g + 1) * KCG)
            nc.scalar.copy(st32[:, gs], St[:])
            nc.scalar.copy(counts[0:1, gs], Cnt[:])

    ar1 = dram.tile([P + 1, KC], F32)
    ar1o = dram.tile([P + 1, KC], F32)
    nc.sync.dma_start(out=ar1[0:P, :], in_=st32[:])
    nc.sync.dma_start(out=ar1[P:P + 1, :], in_=counts[:])
    nc.gpsimd.collective_compute(
        "AllReduce", ALU.add, replica_groups=[list(range(n_cores))],
        ins=[ar1.opt()], outs=[ar1o.opt()])
    nc.sync.dma_start(out=st32[:], in_=ar1o[0:P, :])
    nc.sync.dma_start(out=counts[:], in_=ar1o[P:P + 1, :])

    # ---- mid: centroids, column norms -----------------------------------
    cmax = rows2.tile([1, KC], F32, tag="rsc")
    nc.vector.tensor_scalar(out=cmax[:], in0=counts[:], scalar1=1.0,
                            scalar2=None, op0=ALU.max)
    invn = rows2.tile([1, KC], F32, tag="rsc")
    nc.vector.reciprocal(invn[:], cmax[:])
    invn16 = rows.tile([1, KC], F16)
    nc.vector.tensor_copy(invn16[:], invn[:])

    Ct16 = big.tile([P, KC], F16)
    mhcn2 = rows.tile([1, KC], F16)
    with tc.tile_pool(name="psM", bufs=1, space="PSUM") as psM:
        for g in range(NG):
            gs = slice(g * KCG, (g + 1) * KCG)
            bc = psM.tile([P, KCG], F32, tag="bc")
            for o, w in _chunks(KCG):
                nc.tensor.matmul(bc[:, o:o + w], ones_row16[:],
                                 invn16[0:1, g * KCG + o:g * KCG + o + w],
                                 start=True, stop=True)
            nc.vector.tensor_tensor(out=Ct16[:, gs], in0=st32[:, gs],
                                    in1=bc[:], op=ALU.mult)
            ctsq = scr.tile([P, KCG], F16, tag="ctsq")
            nc.scalar.activation(out=ctsq[:], in_=Ct16[:, gs], func=ACTF.Square)
            cnp = psM.tile([1, KCG], F32, tag="cnp")
            for o, w in _chunks(KCG):
                nc.tensor.matmul(cnp[0:1, o:o + w], ones_col16[:],
                                 ctsq[:, o:o + w], start=True, stop=True)
            nc.scalar.mul(mhcn2[0:1, gs], cnp[:], -0.5)

    # ---- pass B: per-sample distances -> per-cluster sums ---------------
    persum = rows.tile([1, KC], F32)
    with tc.tile_pool(name="psB", bufs=1, space="PSUM") as psB:
        for g in range(NG):
            PS = psB.tile([1, KCG], F32, tag="ps")
            for n in range(T):
                oh = gen_oh(n, g)
                Gp = psB.tile([P, KCG], F32, tag="g")
                dst = scr.tile([P, KCG], F16, tag="dst")
                ohd = scr.tile([P, KCG], F16, tag="ohd")
                for o, w in _chunks(KCG):
                    co = g * KCG + o
                    nc.tensor.matmul(Gp[:, o:o + w],
                                     fT16[:, n * D:(n + 1) * D],
                                     Ct16[:, co:co + w],
                                     start=True, stop=False)
                    nc.tensor.matmul(Gp[:, o:o + w], ones_row16[:],
                                     mhcn2[0:1, co:co + w],
                                     start=False, stop=True)
                    nc.scalar.activation(
                        out=dst[:, o:o + w], in_=Gp[:, o:o + w],
                        func=ACTF.Sqrt, bias=fnorm[:, n:n + 1], scale=-2.0)
                    nc.vector.tensor_tensor(
                        out=ohd[:, o:o + w], in0=oh[:, o:o + w],
                        in1=dst[:, o:o + w], op=ALU.mult)
                    nc.tensor.matmul(PS[0:1, o:o + w], ones_col16[:],
                                     ohd[:, o:o + w],
                                     start=(n == 0), stop=(n == T - 1))
            nc.scalar.copy(persum[0:1, g * KCG:(g + 1) * KCG], PS[:])

    ar2 = dram.tile([1, KC], F32)
    ar2o = dram.tile([1, KC], F32)
    nc.sync.dma_start(out=ar2[:], in_=persum[:])
    nc.gpsimd.collective_compute(
        "AllReduce", ALU.add, replica_groups=[list(range(n_cores))],
        ins=[ar2.opt()], outs=[ar2o.opt()])
    nc.sync.dma_start(out=persum[:], in_=ar2o[:])

    # ---- final stats (redundant on every core) --------------------------
    # [1,KC] rows -> [NC,K] layout via DRAM bounce
    cb = dram.tile([NC, K], F32)
    pb = dram.tile([NC, K], F32)
    nc.sync.dma_start(out=cb[:].rearrange("c k -> (c k)"), in_=counts[:])
    nc.sync.dma_start(out=pb[:].rearrange("c k -> (c k)"), in_=persum[:])
    counts2 = fin.tile([NC, K], F32)
    persum2 = fin.tile([NC, K], F32)
    nc.sync.dma_start(out=counts2[:], in_=cb[:])
    nc.sync.dma_start(out=persum2[:], in_=pb[:])

    ne2 = fin.tile([NC, K], F32)
    nc.vector.tensor_scalar(out=ne2[:], in0=counts2[:], scalar1=0.0,
                            scalar2=None, op0=ALU.is_gt)
    ne16 = fin.tile([NC, K], F16)
    nc.vector.tensor_copy(ne16[:], ne2[:])
    multi = fin.tile([NC, K], F32)
    nc.vector.tensor_scalar(out=multi[:], in0=counts2[:], scalar1=1.0,
                            scalar2=None, op0=ALU.is_gt)

    cmax2 = fin.tile([NC, K], F32)
    nc.vector.tensor_scalar(out=cmax2[:], in0=counts2[:], scalar1=1.0,
                            scalar2=None, op0=ALU.max)
    invn2 = fin.tile([NC, K], F32)
    nc.vector.reciprocal(invn2[:], cmax2[:])
    per_mean = fin.tile([NC, K], F32)
    nc.vector.tensor_tensor(out=per_mean[:], in0=persum2[:], in1=invn2[:],
                            op=ALU.mult)

    nn = fin.tile([NC, 1], F32)
    nc.vector.tensor_reduce(out=nn[:], in_=ne2[:], axis=mybir.AxisListType.X,
                            op=ALU.add)
    n_multi = fin.tile([NC, 1], F32)
    nc.vector.tensor_reduce(out=n_multi[:], in_=multi[:],
                            axis=mybir.AxisListType.X, op=ALU.add)

    mpm = fin.tile([NC, K], F32)
    nc.vector.tensor_tensor(out=mpm[:], in0=multi[:], in1=per_mean[:],
                            op=ALU.mult)
    hasum = fin.tile([NC, 1], F32)
    nc.vector.tensor_reduce(out=hasum[:], in_=mpm[:],
                            axis=mybir.AxisListType.X, op=ALU.add)
    nmc = fin.tile([NC, 1], F32)
    nc.vector.tensor_scalar(out=nmc[:], in0=n_multi[:], scalar1=1.0,
                            scalar2=None, op0=ALU.max)
    nmi = fin.tile([NC, 1], F32)
    nc.vector.reciprocal(nmi[:], nmc[:])
    h_a = fin.tile([NC, 1], F32)
    nc.vector.tensor_tensor(out=h_a[:], in0=hasum[:], in1=nmi[:], op=ALU.mult)

    multi_m = fin.tile([NC, K], mybir.dt.uint8)
    nc.vector.tensor_copy(multi_m[:], multi[:])
    minpre = fin.tile([NC, K], F32)
    nc.vector.memset(minpre[:], BIG)
    nc.vector.copy_predicated(out=minpre[:], mask=multi_m[:], data=per_mean[:])
    min_intra = fin.tile([NC, 1], F32)
    nc.vector.tensor_reduce(out=min_intra[:], in_=minpre[:],
                            axis=mybir.AxisListType.X, op=ALU.min)

    has_multi = fin.tile([NC, 1], F32)
    nc.vector.tensor_scalar(out=has_multi[:], in0=n_multi[:], scalar1=0.0,
                            scalar2=None, op0=ALU.is_gt)
    nc.vector.tensor_tensor(out=h_a[:], in0=h_a[:], in1=has_multi[:],
                            op=ALU.mult)
    min_intra2 = fin.tile([NC, 1], F32)
    nc.vector.tensor_tensor(out=min_intra2[:], in0=min_intra[:],
                            in1=has_multi[:], op=ALU.mult)

    # entropy
    pp = fin.tile([NC, K], F32)
    nc.vector.tensor_scalar(out=pp[:], in0=counts2[:],
                            scalar1=1.0 / (n_cores * BL),
                            scalar2=1e-10, op0=ALU.mult, op1=ALU.add)
    lnp = fin.tile([NC, K], F32)
    nc.scalar.activation(out=lnp[:], in_=pp[:], func=ACTF.Ln)
    plp = fin.tile([NC, K], F32)
    nc.vector.tensor_tensor(out=plp[:], in0=pp[:], in1=lnp[:], op=ALU.mult)
    hsum = fin.tile([NC, 1], F32)
    nc.vector.tensor_reduce(out=hsum[:], in_=plp[:],
                            axis=mybir.AxisListType.X, op=ALU.add)
    H = fin.tile([NC, 1], F32)
    nc.vector.tensor_scalar(out=H[:], in0=hsum[:], scalar1=-1.0, scalar2=None,
                            op0=ALU.mult)

    # ---- inter-centroid distances ---------------------------------------
    # ne as padded row [1, NC*128] and as column per config [K?, NC]
    neb_d = dram.tile([NC, K], F16)
    nc.sync.dma_start(out=neb_d[:], in_=ne16[:])
    nepad = rows.tile([1, NC * P], F16)
    nc.vector.memset(nepad[:], 0.0)
    nc.sync.dma_start(
        out=nepad[0:1, :].rearrange("p (c k) -> p c k", k=P)[:, :, 0:K],
        in_=neb_d[:])

    necf = fin.tile([K, NC], F32)
    with tc.tile_pool(name="psN", bufs=1, space="PSUM") as psN:
        nps = psN.tile([K, NC], F32)
        nc.tensor.transpose(nps[:], ne2[:], ident32[0:NC, 0:NC])
        nc.scalar.copy(necf[:], nps[:])

    inter16 = big.tile([P, NC * P], F16)
    sums_pc = fin.tile([K, NC], F32)
    maxs_pc = fin.tile([K, NC], F32)
    HNC = NC // NG  # configs per half
    HW = HNC * P    # 2048
    with tc.tile_pool(name="psF", bufs=1, space="PSUM") as psF:
        for h in range(NG):
            d2 = psF.tile([K, HW], F32, tag="d2")
            neb = psF.tile([K, HW], F32, tag="neb")
            nc.vector.memset(d2[:], 0.0)
            for o, w in _chunks(HW):
                nc.tensor.matmul(neb[:, o:o + w], ones_row16[0:1, 0:K],
                                 nepad[0:1, h * HW + o:h * HW + o + w],
                                 start=True, stop=True)
            for j in range(HNC):
                c = h * HNC + j
                sl = slice(c * K, (c + 1) * K)
                blk = slice(j * P, j * P + K)
                nc.tensor.matmul(d2[:, blk], Ct16[:, sl], Ct16[:, sl],
                                 start=True, stop=False)
                nc.tensor.matmul(d2[:, blk], ones_row16[0:1, 0:K],
                                 mhcn2[0:1, sl], start=False, stop=False)
                nc.tensor.matmul(d2[:, blk], mhcn2[0:1, sl],
                                 ones_row16[0:1, 0:K], start=False, stop=True)
            dcl = scr.tile([K, HW], F16, tag="dcl")
            nc.vector.tensor_scalar(out=dcl[:], in0=d2[:], scalar1=-2.0,
                                    scalar2=1e-12, op0=ALU.mult, op1=ALU.max)
            isl = slice(h * HW, (h + 1) * HW)
            nc.scalar.activation(out=inter16[0:K, isl], in_=dcl[:],
                                 func=ACTF.Sqrt)
            for j in range(HNC):
                c = h * HNC + j
                bsl = slice(c * P, c * P + P)
                x = scr.tile([K, P], F16, tag="mask")
                nc.vector.tensor_tensor(out=x[:], in0=inter16[0:K, bsl],
                                        in1=tri16[0:K, :], op=ALU.mult)
                nc.vector.tensor_tensor(out=x[:], in0=x[:],
                                        in1=neb[:, j * P:(j + 1) * P],
                                        op=ALU.mult)
                nc.vector.tensor_scalar(out=inter16[0:K, bsl], in0=x[:],
                                        scalar1=necf[:, c:c + 1], scalar2=None,
                                        op0=ALU.mult)
                nc.vector.tensor_reduce(out=sums_pc[:, c:c + 1],
                                        in_=inter16[0:K, bsl],
                                        axis=mybir.AxisListType.X, op=ALU.add)
                nc.vector.tensor_reduce(out=maxs_pc[:, c:c + 1],
                                        in_=inter16[0:K, bsl],
                                        axis=mybir.AxisListType.X, op=ALU.max)

    sums_t = fin.tile([NC, K], F32)
    maxs_t = fin.tile([NC, K], F32)
    with tc.tile_pool(name="psX", bufs=2, space="PSUM") as psX:
        tp1 = psX.tile([NC, K], F32, tag="tp")
        nc.tensor.transpose(tp1[:], sums_pc[:], ident32[0:K, 0:K])
        nc.scalar.copy(sums_t[:], tp1[:])
        tp2 = psX.tile([NC, K], F32, tag="tp")
        nc.tensor.transpose(tp2[:], maxs_pc[:], ident32[0:K, 0:K])
        nc.scalar.copy(maxs_t[:], tp2[:])

    pairsum = fin.tile([NC, 1], F32)
    nc.vector.tensor_reduce(out=pairsum[:], in_=sums_t[:],
                            axis=mybir.AxisListType.X, op=ALU.add)
    max_inter = fin.tile([NC, 1], F32)
    nc.vector.tensor_reduce(out=max_inter[:], in_=maxs_t[:],
                            axis=mybir.AxisListType.X, op=ALU.max)

    # npair = nn*(nn-1)/2
    nm1 = fin.tile([NC, 1], F32)
    nc.vector.tensor_scalar(out=nm1[:], in0=nn[:], scalar1=-1.0, scalar2=None,
                            op0=ALU.add)
    npair = fin.tile([NC, 1], F32)
    nc.vector.tensor_tensor(out=npair[:], in0=nm1[:], in1=nn[:], op=ALU.mult)
    nc.vector.tensor_scalar(out=npair[:], in0=npair[:], scalar1=0.5,
                            scalar2=None, op0=ALU.mult)
    has_pair = fin.tile([NC, 1], F32)
    nc.vector.tensor_scalar(out=has_pair[:], in0=npair[:], scalar1=0.0,
                            scalar2=None, op0=ALU.is_gt)
    npc = fin.tile([NC, 1], F32)
    nc.vector.tensor_scalar(out=npc[:], in0=npair[:], scalar1=1.0,
                            scalar2=None, op0=ALU.max)
    npi = fin.tile([NC, 1], F32)
    nc.vector.reciprocal(npi[:], npc[:])
    h_r = fin.tile([NC, 1], F32)
    nc.vector.tensor_tensor(out=h_r[:], in0=pairsum[:], in1=npi[:],
                            op=ALU.mult)
    nc.vector.tensor_tensor(out=h_r[:], in0=h_r[:], in1=has_pair[:],
                            op=ALU.mult)
    maxi2 = fin.tile([NC, 1], F32)
    nc.vector.tensor_tensor(out=maxi2[:], in0=max_inter[:], in1=has_pair[:],
                            op=ALU.mult)
    delta = fin.tile([NC, 1], F32)
    nc.vector.tensor_tensor(out=delta[:], in0=maxi2[:], in1=min_intra2[:],
                            op=ALU.subtract)

    many = fin.tile([NC, 1], F32)
    nc.vector.tensor_scalar(out=many[:], in0=nn[:], scalar1=1.0, scalar2=None,
                            op0=ALU.is_gt)
    nc.vector.tensor_tensor(out=h_a[:], in0=h_a[:], in1=many[:], op=ALU.mult)
    nc.vector.tensor_tensor(out=h_r[:], in0=h_r[:], in1=many[:], op=ALU.mult)
    nc.vector.tensor_tensor(out=delta[:], in0=delta[:], in1=many[:],
                            op=ALU.mult)

    # ---- assemble, normalize, broadcast out -----------------------------
    e = fin.tile([NC, 4], F32)
    nc.vector.tensor_copy(e[:, 0:1], H[:])
    nc.vector.tensor_copy(e[:, 1:2], h_a[:])
    nc.vector.tensor_copy(e[:, 2:3], h_r[:])
    nc.vector.tensor_copy(e[:, 3:4], delta[:])

    rm = fin.tile([NC, 4], F32)
    nc.sync.dma_start(out=rm[:], in_=rm_d.ap())
    rv = fin.tile([NC, 4], F32)
    nc.sync.dma_start(out=rv[:], in_=rv_d.ap())
    sqv = fin.tile([NC, 4], F32)
    nc.scalar.activation(out=sqv[:], in_=rv[:], func=ACTF.Sqrt)
    nc.vector.tensor_scalar(out=sqv[:], in0=sqv[:], scalar1=1e-8, scalar2=None,
                            op0=ALU.add)
    deni = fin.tile([NC, 4], F32)
    nc.vector.reciprocal(deni[:], sqv[:])
    enorm = fin.tile([NC, 4], F32)
    nc.vector.tensor_tensor(out=enorm[:], in0=e[:], in1=rm[:], op=ALU.subtract)
    nc.vector.tensor_tensor(out=enorm[:], in0=enorm[:], in1=deni[:],
                            op=ALU.mult)

    eb_d = dram.tile([NC, 4], F32)
    nc.sync.dma_start(out=eb_d[:], in_=enorm[:])
    erow = fin.tile([1, NC * 4], F32)
    nc.sync.dma_start(out=erow[:], in_=eb_d[:].rearrange("c k -> (c k)"))

    eout = fin.tile([P, NC * 4], F32)
    with tc.tile_pool(name="psO", bufs=1, space="PSUM") as psO:
        ebps = psO.tile([P, NC * 4], F32)
        nc.tensor.matmul(ebps[:], ones_row32[:], erow[:], start=True, stop=True)
        nc.scalar.copy(eout[:], ebps[:])
    outv = out_d.ap().rearrange("(r p) q -> r p q", p=P)
    for r in range(T):
        nc.sync.dma_start(out=outv[r], in_=eout[:])


_PROG_CACHE = {}


def build_program(BL=B // 8, n_cores=8, q_eps=0.0):
    key = (BL, n_cores, q_eps)
    if key in _PROG_CACHE:
        return _PROG_CACHE[key]
    nc = bacc.Bacc("TRN2", target_bir_lowering=False, debug=False,
                   num_devices=n_cores)
    with tile.TileContext(nc) as tc, ExitStack() as ctx:
        _emit(tc, ctx, n_cores, BL, q_eps=q_eps)
    nc.compile()
    _PROG_CACHE[key] = nc
    return nc


def kernel(features, cluster_assignments, running_mean, running_var):
    n_cores = 8
    BL = B // n_cores
    feat = np.ascontiguousarray(np.asarray(features, dtype=np.float32))
    a32 = np.ascontiguousarray(np.asarray(cluster_assignments, dtype=np.int32))
    rm = np.ascontiguousarray(np.asarray(running_mean, dtype=np.float32))
    rv = np.ascontiguousarray(np.asarray(running_var, dtype=np.float32))

    nc = build_program(BL, n_cores)
    in_maps = [{
        "features": feat[c * BL:(c + 1) * BL],
        "assign": a32[c * BL:(c + 1) * BL],
        "rmean": rm,
        "rvar": rv,
    } for c in range(n_cores)]
    res = run_bass_kernel_spmd(nc, in_maps, core_ids=list(range(n_cores)))
    out = np.concatenate([res.results[c]["out"] for c in range(n_cores)],
                         axis=0)
    return out.reshape(B, NC, 4).astype(np.float32)
